# revision 1
# baseline (speedup 1.0000x reference)
"""ChildSum TreeGRU on 8 Trainium2 NeuronCores.

Data-parallel over trees (16 trees/core). On-device layout is feature-major
([256 feat] -> 2x128 partitions, nodes on the free dim); the host transposes
x's leaf slice in and the output back out. All matmuls run as float32r.

Heap tree, per-core column order is tree-major: col = tree*len + in-level pos.
Levels 10(leaves)..6 are processed per group of 4 trees; level-6 results land
in a joint buffer [128, 16*127] covering heap nodes 0..126 (levels 0..6) for
all 16 trees; levels 5..0 are then processed jointly and the buffer is DMA'd
out in one shot.
"""
import sys

for p in ("/opt/trn_rl_repo", "/root/.axon_site/_ro/trn_rl_repo"):
    if p not in sys.path:
        sys.path.insert(0, p)

import numpy as np
import concourse.tile as tile
from concourse import bacc, mybir
from concourse.bass_utils import run_bass_kernel_spmd

f32 = mybir.dt.float32
f32r = mybir.dt.float32r
AF = mybir.ActivationFunctionType
ALU = mybir.AluOpType

T, DEPTH, NN, H = 128, 11, 2047, 256
NCORES = 8
TPC = T // NCORES          # 16 trees per core
G = 2                      # trees per group
NG = TPC // G              # 4 groups
NLEAF = 1 << (DEPTH - 1)   # 1024
LEAF0 = NLEAF - 1          # 1023
JN = (1 << 7) - 1          # 127 nodes/tree in the joint buffer (levels 0..6)
PS_COLS = 1024             # psum batch (2 banks) consumed by one ACT


def _emit_level(nc, P, tag, NT, Lct, hc3, hc_flat, out3, Wt, bias):
    """One GRU level for NT trees with Lct children per tree.

    hc3:    child-state AP [128, NT, Lct] per half (f32r)
    hc_flat: contiguous 2D view [128, NT*Lct] per half, or None (jbuf)
    out3:   output AP [128, NT, Lpt] per half (f32r)
    """
    Lc = NT * Lct
    Lp = Lc // 2
    uzT, urT, ucT = Wt["uz"], Wt["ur"], Wt["uc"]
    bz, br, bc = bias["bz"], bias["br"], bias["bc"]

    def mm_into(ps, off, lhs, rhs_pair):
        # accumulate both K-halves of one <=512-col chunk into ps[:, off:...]
        n = rhs_pair[0].free_size()
        nc.tensor.matmul(ps[:, off:off + n], lhs[0], rhs_pair[0], start=True, stop=False)
        nc.tensor.matmul(ps[:, off:off + n], lhs[1], rhs_pair[1], start=False, stop=True)

    def child_chunks():
        # yield (cols_off, [rhs_half0, rhs_half1]) chunks of <=512 child cols
        if hc_flat is not None:
            for c0 in range(0, Lc, 512):
                n = min(512, Lc - c0)
                yield c0, [hc_flat[k][:, c0:c0 + n] for k in range(2)]
        else:
            tch = max(1, 512 // Lct)
            for t0 in range(0, NT, tch):
                t1 = min(NT, t0 + tch)
                yield t0 * Lct, [hc3[k][:, t0:t1, :] for k in range(2)]

    # --- h_sum = hc_even + hc_odd (strided), f32r; r-path emitted first so
    # the serial r -> rh -> Uc -> hcand chain starts as early as possible ---
    hs = [P["hs"].tile([128, Lp], f32r, name=f"hs{tag}_{m}", tag=f"hs{m}") for m in range(2)]
    for m in range(2):
        nc.vector.tensor_tensor(hs[m][:], hc3[m][:, :, 0::2], hc3[m][:, :, 1::2], ALU.add)

    # --- r = sigmoid(Ur @ h_sum + br) ---
    r = [P["r"].tile([128, Lp], f32, name=f"r{tag}_{m}", tag=f"r{m}") for m in range(2)]
    for m in range(2):
        lhs = [urT[k][:, m * 128:(m + 1) * 128] for k in range(2)]
        for p0 in range(0, Lp, PS_COLS):
            pn = min(PS_COLS, Lp - p0)
            ps = P["psrc"].tile([128, pn], f32, name=f"psr{tag}_{m}_{p0}", tag="psrc")
            for c0 in range(p0, p0 + pn, 512):
                n = min(512, p0 + pn - c0)
                mm_into(ps, c0 - p0, lhs, [hs[k][:, c0:c0 + n] for k in range(2)])
            nc.scalar.activation(r[m][:, p0:p0 + pn], ps[:], AF.Sigmoid, bias=br[m][:])

    # --- z = sigmoid(Uz @ hc + bz) over all children (fills PE while ACT r runs) ---
    z = [P["z"].tile([128, Lc], f32, name=f"z{tag}_{m}", tag=f"z{m}") for m in range(2)]
    for m in range(2):
        lhs = [uzT[k][:, m * 128:(m + 1) * 128] for k in range(2)]
        for p0 in range(0, Lc, PS_COLS):
            pn = min(PS_COLS, Lc - p0)
            ps = P["psz"].tile([128, pn], f32, name=f"psz{tag}_{m}_{p0}", tag="psz")
            for c0, rhs in child_chunks():
                if p0 <= c0 < p0 + pn:
                    mm_into(ps, c0 - p0, lhs, rhs)
            nc.scalar.activation(z[m][:, p0:p0 + pn], ps[:], AF.Sigmoid, bias=bz[m][:])

    # --- rh = r * h_sum (in place into hs, stays f32r) ---
    for m in range(2):
        nc.vector.tensor_tensor(hs[m][:], r[m][:], hs[m][:], ALU.mult)

    # --- h_cand = tanh(Uc @ rh + bc) ---
    hcand = [P["hc"].tile([128, Lp], f32, name=f"hcand{tag}_{m}", tag=f"hcand{m}") for m in range(2)]
    for m in range(2):
        lhs = [ucT[k][:, m * 128:(m + 1) * 128] for k in range(2)]
        for p0 in range(0, Lp, PS_COLS):
            pn = min(PS_COLS, Lp - p0)
            ps = P["psrc"].tile([128, pn], f32, name=f"psc{tag}_{m}_{p0}", tag="psrc")
            for c0 in range(p0, p0 + pn, 512):
                n = min(512, p0 + pn - c0)
                mm_into(ps, c0 - p0, lhs, [hs[k][:, c0:c0 + n] for k in range(2)])
            nc.scalar.activation(hcand[m][:, p0:p0 + pn], ps[:], AF.Tanh, bias=bc[m][:])

    for m in range(2):
        z3 = z[m][:].rearrange("p (t n) -> p t n", t=NT)
        # zs = z_even + z_odd  (before z is overwritten by zh); reuses the r slot
        zs = P["r"].tile([128, Lp], f32, name=f"zs{tag}_{m}", tag=f"r{m}")
        nc.vector.tensor_tensor(zs[:], z3[:, :, 0::2], z3[:, :, 1::2], ALU.add)
        # zh = z * hc, in place into z (DVE: gpsimd would contend for the
        # shared DVE/GpSimd SBUF port pair and slow both engines ~4x)
        nc.vector.tensor_tensor(z[m][:], z[m][:], hc3[m].bitcast(f32), ALU.mult)
        # zh_sum = zh_even + zh_odd; reuses the h_sum slot
        zhs = P["hs"].tile([128, Lp], f32, name=f"zhs{tag}_{m}", tag=f"hs{m}")
        nc.vector.tensor_tensor(zhs[:], z3[:, :, 0::2], z3[:, :, 1::2], ALU.add)
        # t = (zs - 1) * h_cand, in place into hcand
        nc.vector.scalar_tensor_tensor(hcand[m][:], zs[:], 1.0, hcand[m][:], ALU.subtract, ALU.mult)
        # h_new = zh_sum - t  -> out3 (f32r)
        nc.vector.tensor_tensor(out3[m], zhs[:], hcand[m][:], ALU.subtract)


def _build():
    nc = bacc.Bacc("TRN2", debug=False)

    xT_d = nc.dram_tensor("xT", [H, TPC * NLEAF], f32r, kind="ExternalInput")
    wT_d = nc.dram_tensor("wT", [H, H], f32r, kind="ExternalInput")
    uzT_d = nc.dram_tensor("uzT", [H, H], f32r, kind="ExternalInput")
    urT_d = nc.dram_tensor("urT", [H, H], f32r, kind="ExternalInput")
    ucT_d = nc.dram_tensor("ucT", [H, H], f32r, kind="ExternalInput")
    bw_d = nc.dram_tensor("bw", [H, 1], f32, kind="ExternalInput")
    bz_d = nc.dram_tensor("bz", [H, 1], f32, kind="ExternalInput")
    br_d = nc.dram_tensor("br", [H, 1], f32, kind="ExternalInput")
    bc_d = nc.dram_tensor("bc", [H, 1], f32, kind="ExternalInput")
    hout_d = nc.dram_tensor("h_out", [H, TPC, NN], f32, kind="ExternalOutput")

    with tile.TileContext(nc) as tc:
        from contextlib import ExitStack
        with ExitStack() as ctx:
            P = {}
            P["const"] = ctx.enter_context(tc.tile_pool(name="const", bufs=1))
            P["xg"] = ctx.enter_context(tc.tile_pool(name="xg", bufs=2))
            P["h10"] = ctx.enter_context(tc.tile_pool(name="h10", bufs=2))
            P["hl"] = ctx.enter_context(tc.tile_pool(name="hl", bufs=2))
            P["jbuf"] = ctx.enter_context(tc.tile_pool(name="jbuf", bufs=1))
            P["z"] = ctx.enter_context(tc.tile_pool(name="z", bufs=2))
            P["hs"] = ctx.enter_context(tc.tile_pool(name="hs", bufs=2))
            P["r"] = ctx.enter_context(tc.tile_pool(name="r", bufs=2))
            P["hc"] = ctx.enter_context(tc.tile_pool(name="hc", bufs=2))
            P["psz"] = ctx.enter_context(tc.tile_pool(name="psz", bufs=2, space="PSUM"))
            P["psrc"] = ctx.enter_context(tc.tile_pool(name="psrc", bufs=2, space="PSUM"))

            cp = P["const"]
            Wt = {}
            for nm, d in (("w", wT_d), ("uz", uzT_d), ("ur", urT_d), ("uc", ucT_d)):
                Wt[nm] = [cp.tile([128, H], f32r, name=f"{nm}T{k}") for k in range(2)]
                for k in range(2):
                    nc.sync.dma_start(Wt[nm][k][:], d.ap()[k * 128:(k + 1) * 128, :])
            bias = {}
            for nm, d in (("bw", bw_d), ("bz", bz_d), ("br", br_d), ("bc", bc_d)):
                bias[nm] = [cp.tile([128, 1], f32, name=f"{nm}{m}") for m in range(2)]
                for m in range(2):
                    nc.sync.dma_start(bias[nm][m][:], d.ap()[m * 128:(m + 1) * 128, :])

            # joint buffer: heap nodes 0..126 for all 16 trees, per half
            jbuf = [P["jbuf"].tile([128, TPC * JN], f32r, name=f"jbuf{m}") for m in range(2)]
            jv = [jbuf[m][:].rearrange("p (t n) -> p t n", t=TPC) for m in range(2)]

            def emit_leaf(g):
                gt = f"g{g}"
                xg = [P["xg"].tile([128, G * NLEAF], f32r, name=f"x{gt}_{k}", tag="xg")
                      for k in range(2)]
                for k in range(2):
                    for piece in range(0, G * NLEAF, 1024):
                        pend = min(piece + 1024, G * NLEAF)
                        nc.sync.dma_start(
                            xg[k][:, piece:pend],
                            xT_d.ap()[k * 128:(k + 1) * 128,
                                      g * G * NLEAF + piece:g * G * NLEAF + pend])
                h10 = [P["h10"].tile([128, G * NLEAF], f32r, name=f"h10{gt}_{m}", tag=f"h10{m}")
                       for m in range(2)]
                for m in range(2):
                    lhs = [Wt["w"][k][:, m * 128:(m + 1) * 128] for k in range(2)]
                    for p0 in range(0, G * NLEAF, PS_COLS):
                        pn = min(PS_COLS, G * NLEAF - p0)
                        ps = P["psz"].tile([128, pn], f32, name=f"psx{gt}_{m}_{p0}", tag="psz")
                        for c0 in range(p0, p0 + pn, 512):
                            n = min(512, p0 + pn - c0)
                            nc.tensor.matmul(ps[:, c0 - p0:c0 - p0 + n], lhs[0],
                                             xg[0][:, c0:c0 + n], start=True, stop=False)
                            nc.tensor.matmul(ps[:, c0 - p0:c0 - p0 + n], lhs[1],
                                             xg[1][:, c0:c0 + n], start=False, stop=True)
                        nc.scalar.activation(h10[m][:, p0:p0 + pn], ps[:], AF.Tanh,
                                             bias=bias["bw"][m][:])
                    nc.sync.dma_start(
                        hout_d.ap()[m * 128:(m + 1) * 128, g * G:(g + 1) * G,
                                    LEAF0:LEAF0 + NLEAF],
                        h10[m][:].rearrange("p (t n) -> p t n", t=G).bitcast(f32))
                return h10

            def emit_lvl(g, lv, hchild):
                gt = f"g{g}"
                Lct = 2 ** (lv + 1)
                Lpt = 2 ** lv
                hc3 = [hchild[m][:].rearrange("p (t n) -> p t n", t=G) for m in range(2)]
                hc_flat = [hchild[m][:] for m in range(2)]
                if lv == 6:
                    out3 = [jv[m][:, g * G:(g + 1) * G, Lpt - 1:2 * Lpt - 1]
                            for m in range(2)]
                    hnew = None
                else:
                    hnew = [P["hl"].tile([128, G * Lpt], f32r,
                                         name=f"h{lv}{gt}_{m}", tag=f"h{lv}_{m}")
                            for m in range(2)]
                    out3 = [hnew[m][:].rearrange("p (t n) -> p t n", t=G)
                            for m in range(2)]
                _emit_level(nc, P, f"{gt}l{lv}", G, Lct, hc3, hc_flat, out3, Wt, bias)
                if lv > 6:
                    for m in range(2):
                        nc.sync.dma_start(
                            hout_d.ap()[m * 128:(m + 1) * 128, g * G:(g + 1) * G,
                                        Lpt - 1:2 * Lpt - 1],
                            hnew[m][:].rearrange("p (t n) -> p t n", t=G).bitcast(f32))
                return hnew

            # wavefront: stage s of group g is emitted at tick t = g + s
            # (stage 0 = leaf, stages 1..4 = levels 9..6) so PE always has a
            # dense leaf/z matmul stream while DVE/ACT work the gate math
            gstate = {}
            for t in range(NG + 4):
                for g in range(NG):
                    s = t - g
                    if s < 0 or s > 4:
                        continue
                    if s == 0:
                        gstate[g] = emit_leaf(g)
                    else:
                        gstate[g] = emit_lvl(g, 10 - s, gstate[g])

            # l6 region of the joint buffer is complete: stream it out
            for m in range(2):
                nc.sync.dma_start(
                    hout_d.ap()[m * 128:(m + 1) * 128, :, 63:JN],
                    jv[m][:, :, 63:JN].bitcast(f32))

            # ---- joint levels 5..0 over jbuf, streaming each level out ----
            for m in range(2):
                nc.sync.dma_start(
                    hout_d.ap()[m * 128:(m + 1) * 128, :, 63:JN],
                    jv[m][:, :, 63:JN].bitcast(f32))
            for lv in range(5, -1, -1):
                Lct = 2 ** (lv + 1)
                Lpt = 2 ** lv
                hc3 = [jv[m][:, :, Lct - 1:2 * Lct - 1] for m in range(2)]
                out3 = [jv[m][:, :, Lpt - 1:2 * Lpt - 1] for m in range(2)]
                _emit_level(nc, P, f"j{lv}", TPC, Lct, hc3, None, out3, Wt, bias)
                for m in range(2):
                    nc.sync.dma_start(
                        hout_d.ap()[m * 128:(m + 1) * 128, :, Lpt - 1:2 * Lpt - 1],
                        jv[m][:, :, Lpt - 1:2 * Lpt - 1].bitcast(f32))

    nc.compile()
    return nc


_NC = None


def _get_nc():
    global _NC
    if _NC is None:
        _NC = _build()
    return _NC


def make_in_maps(inputs):
    x = np.asarray(inputs["x"], np.float32)
    W = np.asarray(inputs["W"], np.float32)
    bW = np.asarray(inputs["bW"], np.float32).reshape(H, 1)
    Ur = np.asarray(inputs["Ur"], np.float32)
    br = np.asarray(inputs["br"], np.float32).reshape(H, 1)
    Uc = np.asarray(inputs["Uc"], np.float32)
    bc = np.asarray(inputs["bc"], np.float32).reshape(H, 1)
    Uz = np.asarray(inputs["Uz"], np.float32)
    bz = np.asarray(inputs["bz"], np.float32).reshape(H, 1)
    shared = {
        "wT": np.ascontiguousarray(W.T), "uzT": np.ascontiguousarray(Uz.T),
        "urT": np.ascontiguousarray(Ur.T), "ucT": np.ascontiguousarray(Uc.T),
        "bw": bW, "bz": bz, "br": br, "bc": bc,
    }
    in_maps = []
    for c in range(NCORES):
        xs = x[c * TPC:(c + 1) * TPC, LEAF0:, :]          # [16, 1024, 256]
        xTc = np.ascontiguousarray(xs.transpose(2, 0, 1)).reshape(H, TPC * NLEAF)
        in_maps.append({"xT": xTc, **shared})
    return in_maps


def assemble_out(core_outs):
    out = np.empty((T, NN, H), np.float32)
    for c in range(NCORES):
        # [256, 16, 2047] -> [16, 2047, 256]
        out[c * TPC:(c + 1) * TPC] = core_outs[c].transpose(1, 2, 0)
    return out


def kernel(**inputs):
    nc = _get_nc()
    in_maps = make_in_maps(inputs)
    res = run_bass_kernel_spmd(nc, in_maps, list(range(NCORES)))
    return assemble_out([r["h_out"] for r in res.results])



# revision 4
# speedup vs baseline: 1.3387x; 1.3387x over previous
"""ChildSum TreeGRU on 8 Trainium2 NeuronCores.

Data-parallel over trees (16/core). fp16 on-chip; feature dim folded as
[128 partitions, 2 k-halves, cols]; within each tree level, nodes are stored
in bit-reversed order so children of the (bit-reversed-ordered) parents form
two contiguous blocks [left | right] -> every DVE op is packed/contiguous and
runs in the 2-byte 2x mode. GpSimd takes the z-sum path; ACT ops are fused
across both feature halves (2048 cols) when all biases are zero.

Levels 10(leaf)..6 run per group of 4 trees (wavefront across 4 groups);
level-6 results land in a joint buffer holding levels 0..6 for all 16 trees;
levels 5..0 are processed jointly. Host pre-permutes x and post-permutes the
output (bit-reversal + layout), which is free w.r.t. HW exec time.
"""
import sys

for p in ("/opt/trn_rl_repo", "/root/.axon_site/_ro/trn_rl_repo"):
    if p not in sys.path:
        sys.path.insert(0, p)

import numpy as np
import concourse.tile as tile
from concourse import bacc, mybir
from concourse.bass_utils import run_bass_kernel_spmd

f32 = mybir.dt.float32
fp16 = mybir.dt.float16
AF = mybir.ActivationFunctionType
ALU = mybir.AluOpType

T, DEPTH, NN, H = 128, 11, 2047, 256
NCORES = 8
TPC = T // NCORES          # 16 trees per core
G = 4                      # trees per group
NG = TPC // G              # 4 groups
NLEAF = 1 << (DEPTH - 1)   # 1024

CH_MM = 512                # matmul / psum chunk (one psum bank per half)
CH_V = 1024                # DVE chunk (free size 2048)
CH_G = 512                 # gpsimd chunk
GPS_MIN = 512              # use gpsimd for zs1/zhs when C >= this

# h_out column blocks (device layout, fp16). Group levels 10..7 then jbuf.
OFF10 = 0
OFF9 = OFF10 + TPC * 1024
OFF8 = OFF9 + TPC * 512
OFF7 = OFF8 + TPC * 256
OFFJ = OFF7 + TPC * 128            # 30720
JN = 127                            # nodes/tree in jbuf (levels 0..6)
OUTCOLS = OFFJ + JN * TPC           # 32752
# jbuf block start (in slots) for level l: levels 6,5,...,0 packed high->low
JOFF = {l: (JN - ((1 << (l + 1)) - 1)) for l in range(7)}


def _sig(l):
    s = np.zeros(1, np.int64)
    for _ in range(l):
        s = np.concatenate([2 * s, 2 * s + 1])
    return s


SIG = {l: _sig(l) for l in range(DEPTH)}


def _v3(t):
    """[128, 2*N] tile -> [128, 2, N] view."""
    return t[:].rearrange("p (a b) -> p a b", a=2)


def _gate(nc, P, wt3, rhs3, cols, out_sl, func, bias2, nm, rh_into=None, r_pool=None):
    """out = func(U @ rhs + b) over `cols` parent/child columns.

    rhs3: [128, 2, cols] fp16 view. out_sl(c0, n) -> output AP slice.
    If rh_into is given (r-gate), each chunk's result is multiplied into
    rh_into (in-place rh = r * hs) right after its activation.
    """
    for c0 in range(0, cols, CH_MM):
        n = min(CH_MM, cols - c0)
        ps = P["ps"].tile([128, 2, CH_MM], f32, name=f"ps{nm}_{c0}", tag="ps")
        for m in range(2):
            for k in range(2):
                nc.tensor.matmul(ps[:, m, :n], wt3[:, k, m * 128:(m + 1) * 128],
                                 rhs3[:, k, c0:c0 + n], start=(k == 0), stop=(k == 1))
        if rh_into is None:
            if bias2 is None:
                nc.scalar.activation(out_sl(c0, n), ps[:, :, :n], func)
            else:
                for m in range(2):
                    nc.scalar.activation(out_sl(c0, n, m), ps[:, m, :n], func,
                                         bias=bias2[m][:])
        else:
            rc = r_pool.tile([128, 2, CH_MM], fp16, name=f"r{nm}_{c0}", tag="r")
            if bias2 is None:
                nc.scalar.activation(rc[:, :, :n], ps[:, :, :n], func)
            else:
                for m in range(2):
                    nc.scalar.activation(rc[:, m, :n], ps[:, m, :n], func,
                                         bias=bias2[m][:])
            # rh = r * hs (in place into hs chunk)
            nc.vector.tensor_tensor(rh_into[:, :, c0:c0 + n], rc[:, :, :n],
                                    rh_into[:, :, c0:c0 + n], ALU.mult)


def _emit_level(nc, P, nm, NT, lv, hc3, out_sl, out_max_chunk, Wt, bias, fuse):
    """One GRU level: children hc3 [128, 2, 2C] -> parents via out_sl."""
    C = NT * (1 << lv)
    bz = None if fuse else bias["bz"]
    br = None if fuse else bias["br"]
    bc = None if fuse else bias["bc"]

    # hs = left + right children (packed 2x DVE)
    hs = P["hs"].tile([128, 2, C], fp16, name=f"hs{nm}", tag="hs")
    for c0 in range(0, C, CH_V):
        n = min(CH_V, C - c0)
        nc.vector.tensor_tensor(hs[:, :, c0:c0 + n], hc3[:, :, c0:c0 + n],
                                hc3[:, :, C + c0:C + c0 + n], ALU.add)

    # z = sigmoid(Uz @ hc + bz) over all 2C children
    z = P["z"].tile([128, 2, 2 * C], fp16, name=f"z{nm}", tag="z")
    _gate(nc, P, Wt["uz"], hc3, 2 * C,
          (lambda c0, n, m=None: z[:, :, c0:c0 + n] if m is None
           else z[:, m, c0:c0 + n]),
          AF.Sigmoid, bz, f"z{nm}")

    # zs1 = z_l + z_r (gpsimd TT when big), then -1 in place (DVE 4x)
    zs1 = P["zs1"].tile([128, 2, C], fp16, name=f"zs1{nm}", tag="zs1")
    eng_s = nc.gpsimd if C >= GPS_MIN else nc.vector
    for c0 in range(0, C, CH_G):
        n = min(CH_G, C - c0)
        eng_s.tensor_tensor(zs1[:, :, c0:c0 + n], z[:, :, c0:c0 + n],
                            z[:, :, C + c0:C + c0 + n], ALU.add)
    for c0 in range(0, C, CH_V):
        n = min(CH_V, C - c0)
        nc.vector.tensor_scalar(zs1[:, :, c0:c0 + n], zs1[:, :, c0:c0 + n],
                                -1.0, None, ALU.add)

    # r chain: r = sigmoid(Ur @ hs + br); rh = r * hs in place per chunk
    _gate(nc, P, Wt["ur"], hs[:, :, :], C, None, AF.Sigmoid, br, f"r{nm}",
          rh_into=hs, r_pool=P["r"])

    # h_cand = tanh(Uc @ rh + bc)
    hcand = P["hc"].tile([128, 2, C], fp16, name=f"hc{nm}", tag="hc")
    _gate(nc, P, Wt["uc"], hs[:, :, :], C,
          (lambda c0, n, m=None: hcand[:, :, c0:c0 + n] if m is None
           else hcand[:, m, c0:c0 + n]),
          AF.Tanh, bc, f"c{nm}")

    # zh = z * hc in place (DVE 2x)
    for c0 in range(0, 2 * C, CH_V):
        n = min(CH_V, 2 * C - c0)
        nc.vector.tensor_tensor(z[:, :, c0:c0 + n], z[:, :, c0:c0 + n],
                                hc3[:, :, c0:c0 + n], ALU.mult)

    # zhs = zh_l + zh_r into hs slot (gpsimd when big; hs free after c-matmul)
    eng_a = nc.gpsimd if C >= GPS_MIN else nc.vector
    for c0 in range(0, C, CH_G):
        n = min(CH_G, C - c0)
        eng_a.tensor_tensor(hs[:, :, c0:c0 + n], z[:, :, c0:c0 + n],
                            z[:, :, C + c0:C + c0 + n], ALU.add)

    # t = zs1 * hcand (in place); h_new = zhs - t
    for c0 in range(0, C, CH_V):
        n = min(CH_V, C - c0)
        nc.vector.tensor_tensor(hcand[:, :, c0:c0 + n], zs1[:, :, c0:c0 + n],
                                hcand[:, :, c0:c0 + n], ALU.mult)
    step = min(out_max_chunk, CH_V)
    for c0 in range(0, C, step):
        n = min(step, C - c0)
        nc.vector.tensor_tensor(out_sl(c0, n), hs[:, :, c0:c0 + n],
                                hcand[:, :, c0:c0 + n], ALU.subtract)


def _build(fuse):
    nc = bacc.Bacc("TRN2", debug=False)

    xT_d = nc.dram_tensor("xT", [H, TPC * NLEAF], fp16, kind="ExternalInput")
    wd = {}
    for gnm in ("w", "uz", "ur", "uc"):
        wd[gnm] = nc.dram_tensor(f"{gnm}T", [H, H], fp16, kind="ExternalInput")
    bd = {}
    if not fuse:
        for bnm in ("bw", "bz", "br", "bc"):
            bd[bnm] = nc.dram_tensor(bnm, [H, 1], f32, kind="ExternalInput")
    hout_d = nc.dram_tensor("h_out", [H, OUTCOLS], fp16, kind="ExternalOutput")

    with tile.TileContext(nc) as tc:
        from contextlib import ExitStack
        with ExitStack() as ctx:
            P = {}
            P["const"] = ctx.enter_context(tc.tile_pool(name="const", bufs=1))
            P["xg"] = ctx.enter_context(tc.tile_pool(name="xg", bufs=4))
            P["h10"] = ctx.enter_context(tc.tile_pool(name="h10", bufs=2))
            P["h9"] = ctx.enter_context(tc.tile_pool(name="h9", bufs=2))
            P["h8"] = ctx.enter_context(tc.tile_pool(name="h8", bufs=2))
            P["h7"] = ctx.enter_context(tc.tile_pool(name="h7", bufs=2))
            P["jbuf"] = ctx.enter_context(tc.tile_pool(name="jbuf", bufs=1))
            P["hs"] = ctx.enter_context(tc.tile_pool(name="hs", bufs=2))
            P["r"] = ctx.enter_context(tc.tile_pool(name="r", bufs=4))
            P["hc"] = ctx.enter_context(tc.tile_pool(name="hc", bufs=2))
            P["z"] = ctx.enter_context(tc.tile_pool(name="z", bufs=2))
            P["zs1"] = ctx.enter_context(tc.tile_pool(name="zs1", bufs=2))
            P["ps"] = ctx.enter_context(tc.tile_pool(name="ps", bufs=4, space="PSUM"))

            cp = P["const"]
            Wt = {}
            for gnm in ("w", "uz", "ur", "uc"):
                wtile = cp.tile([128, 2, H], fp16, name=f"{gnm}T")
                Wt[gnm] = wtile
                for k in range(2):
                    nc.sync.dma_start(wtile[:, k, :],
                                      wd[gnm].ap()[k * 128:(k + 1) * 128, :])
            bias = {}
            if not fuse:
                for bnm in ("bw", "bz", "br", "bc"):
                    bias[bnm] = [cp.tile([128, 1], f32, name=f"{bnm}{m}")
                                 for m in range(2)]
                    for m in range(2):
                        nc.sync.dma_start(bias[bnm][m][:],
                                          bd[bnm].ap()[m * 128:(m + 1) * 128, :])
            bw2 = None if fuse else bias["bw"]

            jb = P["jbuf"].tile([128, 2, JN * TPC], fp16, name="jbuf")

            def jview(l):
                a = JOFF[l] * TPC
                return jb[:, :, a:a + (1 << l) * TPC]

            # --- group phase -------------------------------------------------
            def load_x(g):
                ts = []
                for q in range(2):
                    xt = P["xg"].tile([128, 2, 2048], fp16,
                                      name=f"x{g}_{q}", tag="xg")
                    for k in range(2):
                        c0 = g * G * NLEAF + q * 2048
                        nc.sync.dma_start(xt[:, k, :],
                                          xT_d.ap()[k * 128:(k + 1) * 128,
                                                    c0:c0 + 2048])
                    ts.append(xt)
                return ts

            def emit_leaf(g, xts):
                h10 = P["h10"].tile([128, 2, G * NLEAF], fp16,
                                    name=f"h10g{g}", tag="h10")
                for q in range(2):
                    _gate(nc, P, Wt["w"], xts[q][:, :, :], 2048,
                          (lambda c0, n, m=None, q=q:
                           h10[:, :, q * 2048 + c0:q * 2048 + c0 + n] if m is None
                           else h10[:, m, q * 2048 + c0:q * 2048 + c0 + n]),
                          AF.Tanh, bw2, f"x{g}_{q}")
                for k in range(2):
                    nc.sync.dma_start(
                        hout_d.ap()[k * 128:(k + 1) * 128,
                                    OFF10 + g * 4096:OFF10 + (g + 1) * 4096],
                        h10[:, k, :])
                return h10

            def emit_glevel(g, lv, hchild):
                C = G * (1 << lv)
                if lv == 6:
                    jv6 = jview(6)  # [128, 2, 64*16]
                    out4 = jv6.rearrange("p a (s t) -> p a s t", t=TPC)
                    osl = (lambda c0, n: out4[:, :, :, g * G:(g + 1) * G])
                    hnew = None
                    omax = C  # single chunk
                else:
                    pool = {9: "h9", 8: "h8", 7: "h7"}[lv]
                    hnew = P[pool].tile([128, 2, C], fp16,
                                        name=f"h{lv}g{g}", tag=pool)
                    osl = (lambda c0, n: hnew[:, :, c0:c0 + n])
                    omax = CH_V
                _emit_level(nc, P, f"g{g}l{lv}", G, lv, hchild[:, :, :],
                            osl, omax, Wt, bias, fuse)
                if lv > 6:
                    off = {9: OFF9, 8: OFF8, 7: OFF7}[lv]
                    for k in range(2):
                        nc.sync.dma_start(
                            hout_d.ap()[k * 128:(k + 1) * 128,
                                        off + g * C:off + (g + 1) * C],
                            hnew[:, k, :])
                return hnew

            gstate = {}
            xtiles = {0: load_x(0)}
            for t in range(NG + 5):
                for g in range(NG):
                    s = t - g
                    if s < 0 or s > 4:
                        continue
                    if s == 0:
                        if g + 1 < NG:
                            xtiles[g + 1] = load_x(g + 1)
                        gstate[g] = emit_leaf(g, xtiles.pop(g))
                    else:
                        gstate[g] = emit_glevel(g, 10 - s, gstate[g])

            # level-6 block complete -> stream out
            jv6 = jview(6)
            for k in range(2):
                nc.sync.dma_start(
                    hout_d.ap()[k * 128:(k + 1) * 128,
                                OFFJ + JOFF[6] * TPC:OFFJ + (JOFF[6] + 64) * TPC],
                    jv6[:, k, :])

            # --- joint phase: levels 5..0 over jbuf ---------------------------
            for lv in range(5, -1, -1):
                ov = jview(lv)
                _emit_level(nc, P, f"j{lv}", TPC, lv, jview(lv + 1),
                            (lambda c0, n, ov=ov: ov[:, :, c0:c0 + n]),
                            CH_V, Wt, bias, fuse)
                a = JOFF[lv] * TPC
                for k in range(2):
                    nc.sync.dma_start(
                        hout_d.ap()[k * 128:(k + 1) * 128,
                                    OFFJ + a:OFFJ + a + (1 << lv) * TPC],
                        ov[:, k, :])

    nc.compile()
    return nc


_NC = {}


def _get_nc(fuse=True):
    if fuse not in _NC:
        _NC[fuse] = _build(fuse)
    return _NC[fuse]


def make_in_maps(inputs, fuse):
    x = np.asarray(inputs["x"], np.float32)
    shared = {
        "wT": np.ascontiguousarray(np.asarray(inputs["W"], np.float32).T,
                                   dtype=np.float16),
        "uzT": np.ascontiguousarray(np.asarray(inputs["Uz"], np.float32).T,
                                    dtype=np.float16),
        "urT": np.ascontiguousarray(np.asarray(inputs["Ur"], np.float32).T,
                                    dtype=np.float16),
        "ucT": np.ascontiguousarray(np.asarray(inputs["Uc"], np.float32).T,
                                    dtype=np.float16),
    }
    if not fuse:
        shared.update({
            "bw": np.asarray(inputs["bW"], np.float32).reshape(H, 1),
            "bz": np.asarray(inputs["bz"], np.float32).reshape(H, 1),
            "br": np.asarray(inputs["br"], np.float32).reshape(H, 1),
            "bc": np.asarray(inputs["bc"], np.float32).reshape(H, 1),
        })
    sig10 = SIG[10]
    in_maps = []
    for c in range(NCORES):
        xc = x[c * TPC:(c + 1) * TPC, NLEAF - 1:, :]       # [16, 1024, 256]
        xs = xc[:, sig10, :]                                # slot order
        xT = xs.reshape(NG, G, NLEAF, H).transpose(3, 0, 2, 1).reshape(
            H, TPC * NLEAF)
        in_maps.append({"xT": np.ascontiguousarray(xT, dtype=np.float16),
                        **shared})
    return in_maps


def assemble_out(core_outs):
    out = np.empty((T, NN, H), np.float32)
    for c in range(NCORES):
        ho = np.asarray(core_outs[c])                       # [256, 32752] fp16
        oc = out[c * TPC:(c + 1) * TPC]
        for lv, off in ((10, OFF10), (9, OFF9), (8, OFF8), (7, OFF7)):
            Pl = 1 << lv
            blk = ho[:, off:off + TPC * Pl].reshape(H, NG, Pl, G)
            b = blk.transpose(1, 3, 2, 0).reshape(TPC, Pl, H)
            oc[:, (Pl - 1) + SIG[lv], :] = b.astype(np.float32)
        for lv in range(6, -1, -1):
            Pl = 1 << lv
            a = OFFJ + JOFF[lv] * TPC
            blk = ho[:, a:a + Pl * TPC].reshape(H, Pl, TPC)
            oc[:, (Pl - 1) + SIG[lv], :] = blk.transpose(2, 1, 0).astype(
                np.float32)
    return out


def kernel(**inputs):
    assert int(inputs["depth"]) == DEPTH
    fuse = all(not np.any(np.asarray(inputs[b]))
               for b in ("bW", "br", "bc", "bz"))
    nc = _get_nc(fuse)
    in_maps = make_in_maps(inputs, fuse)
    res = run_bass_kernel_spmd(nc, in_maps, list(range(NCORES)))
    return assemble_out([r["h_out"] for r in res.results])
